# revision 6
# baseline (speedup 1.0000x reference)
"""DSA sparse attention (context-parallel variant) for Trainium2 via Bass/Tile.

Dense-rewrite algorithm (mathematically identical to the reference):
  w[s,t] = exp(sc[s,t])*ts[s,t] / sum_t' exp(sc)*ts   (softmax->*ts->renorm collapses)
  TS[s,j] = sum_t ts[s,t]*[idx[s,t]==j]  -> dense scatter of score values
  E[s,j]  = TS[s,j]*exp(scale*S[s,j]),  S = Q K^T (dense)
  O       = (E @ V) / rowsum(E)
Everything is computed in transposed layout (kv on partitions); O comes out
natural via E^T-stationary matmuls; rowsum(E) falls out of a ones-column
appended to V.

V3 layout/scheduling notes (over V2):
  - host pre-TRANSPOSES q (pre-scaled) and k to [D, S] layout: the on-chip
    PE transposes (80 matmuls) + DVE evacuation copies disappear and the
    q/k DMAs become perfectly contiguous per partition.
  - host pre-builds the DENSE TS table (bf16, [128, NKV, SGRP] per s-group)
    and the kernel DMAs it instead of running 64 gpsimd local_scatters:
    the Pool engine (47us scatters + 11us drains per rep) drops to zero.
  - S psum tiles are [128, 3, 512] (3 banks) so each ACT exp call covers
    1536 elements instead of 1024, amortizing the ~352-cycle ACT pipeline
    fill; EV accumulators shrink to 2 rotating full-bank tiles (the four
    s-blocks are processed in two half-phases of two blocks each), keeping
    total PSUM usage at exactly 8 banks.
  - phases run g-major: (h0,g0) (h1,g0) (h0,g1) (h1,g1); per phase the S^T
    matmuls are WOVEN with the EV matmuls of the previous phase so the PE
    alternates between ACT-gated S work and dependency-free EV work.
"""

import sys

sys.path.insert(0, "/opt/trn_rl_repo")

import numpy as np

import concourse.bass as bass
import concourse.bacc as bacc
import concourse.mybir as mybir
import concourse.tile as tile
from concourse.vector_clock import ScopedClock

# ---------------------------------------------------------------------------
# Patch: this walrus build encodes at most ONE sync-wait on a CTRL NO_STRUCT
# instruction; TileContext's tail drain carries one wait per live proc.  Split
# the waits across a chain of single-wait drains.
# ---------------------------------------------------------------------------


def _patched_drain_and_barrier(self, tick_clock, wait_clock):
    drain_inst = self.nc.sync.drain()
    wait_clock.add_sem_waits(
        drain_inst.ins, ScopedClock({None: tick_clock.global_clock})
    )
    si = drain_inst.ins.sync_info
    if si is not None and len(si.on_wait) > 1:
        waits = list(si.on_wait)
        drain_inst.ins.sync_info = mybir.SyncInfo(
            on_wait=waits[:1], on_update=list(si.on_update)
        )
        for i in range(1, len(waits)):
            extra = self.nc.sync.drain()
            extra.ins.sync_info = mybir.SyncInfo(on_wait=[waits[i]], on_update=[])
    self.nc.all_engine_barrier()
    assert self.sems is not None
    popped = self.nc._tile_sem_poison_stack.pop()
    assert popped is self._sem_poison
    self.nc.clear_and_free_semaphores(list(self.sems.allocated().values()))
    self.nc.all_engine_barrier()


tile.TileContext._drain_and_barrier = _patched_drain_and_barrier

FP = mybir.dt.float32
BF = mybir.dt.bfloat16


class Cfg:
    def __init__(self, HPC=2, SQ=1024, SKV=4096, D=128, TOPK=64):
        self.HPC = HPC  # heads per core
        self.SQ = SQ
        self.SKV = SKV
        self.D = D
        self.TOPK = TOPK
        self.NKV = SKV // 128  # kv chunks of 128
        self.NSB = SQ // 128  # query blocks of 128
        self.SHALF = 512  # s-group width (s-dim per group)
        self.scale = float(D) ** -0.5


# ---------------------------------------------------------------------------
# Program builder
# ---------------------------------------------------------------------------


def build_program(cfg, nmaxs=None, reps=1):
    nc = bacc.Bacc("TRN2", debug=False)
    HPC, SQ, SKV, D, NKV = cfg.HPC, cfg.SQ, cfg.SKV, cfg.D, cfg.NKV
    NGRP = SQ // cfg.SHALF

    qT = nc.dram_tensor("qT", [HPC, D, SQ], BF, kind="ExternalInput").ap()
    kT = nc.dram_tensor("kT", [HPC, D, SKV], BF, kind="ExternalInput").ap()
    v = nc.dram_tensor("v", [HPC, SKV, D], BF, kind="ExternalInput").ap()
    ts = nc.dram_tensor(
        "ts", [NGRP, 128, NKV, cfg.SHALF], BF, kind="ExternalInput"
    ).ap()
    out = nc.dram_tensor("out", [HPC, SQ, D], FP, kind="ExternalOutput").ap()

    with tile.TileContext(nc) as tc:
        import contextlib

        ctx = contextlib.ExitStack()
        with ctx:
            tst_pool = ctx.enter_context(tc.tile_pool(name="tst", bufs=2))
            ktr_pool = ctx.enter_context(tc.tile_pool(name="ktr", bufs=2))
            et_pool = ctx.enter_context(tc.tile_pool(name="et", bufs=2))
            small_pool = ctx.enter_context(tc.tile_pool(name="small", bufs=4))
            out_pool = ctx.enter_context(tc.tile_pool(name="outp", bufs=4))
            s_psum = ctx.enter_context(tc.tile_pool(name="sps", bufs=2, space="PSUM"))
            ev_psum = ctx.enter_context(tc.tile_pool(name="evp", bufs=2, space="PSUM"))

            def _body(_iv=None):
                _build_body(
                    nc, tc, cfg, qT, kT, v, ts, out,
                    tst_pool, ktr_pool, et_pool, small_pool, out_pool,
                    s_psum, ev_psum,
                )

            if reps == 1:
                _body()
            else:
                with tc.For_i(
                    0, reps, 1,
                    hint_engines=(
                        mybir.EngineType.PE,
                        mybir.EngineType.DVE,
                        mybir.EngineType.Activation,
                        mybir.EngineType.Pool,
                        mybir.EngineType.SP,
                    ),
                ):
                    _body()

    nc.compile()
    return nc


def _build_body(nc, tc, cfg, qT, kT, v, ts, out,
                tst_pool, ktr_pool, et_pool, small_pool, out_pool,
                s_psum, ev_psum):
    HPC, SQ, SKV, D, NKV = cfg.HPC, cfg.SQ, cfg.SKV, cfg.D, cfg.NKV
    SGRP = cfg.SHALF
    NGRP = SQ // SGRP
    NSBG = SGRP // 128  # s-blocks per group (4)
    STG = 16            # kv-chunks per v staging DMA

    # ---------------- input DMAs (contiguous, host-prepped layouts) ---------
    def _load_head(h):
        qtr = ktr_pool.tile([128, SQ], BF, tag="qtr")
        nc.sync.dma_start(qtr[:], qT[h])
        ktr = ktr_pool.tile([128, SKV], BF, tag="ktr")
        nc.sync.dma_start(ktr[:, 0 : SKV // 2], kT[h, :, 0 : SKV // 2])
        nc.sync.dma_start(ktr[:, SKV // 2 : SKV], kT[h, :, SKV // 2 : SKV])
        vaug = ktr_pool.tile([128, NKV, D + 1], BF, tag="vaug")
        vview = v[h].rearrange("(n p) d -> p n d", p=128)
        for gdma in range(NKV // STG):
            nc.sync.dma_start(
                vaug[:, gdma * STG : (gdma + 1) * STG, 0:D],
                vview[:, gdma * STG : (gdma + 1) * STG, :],
            )
        nc.vector.memset(vaug[:, :, D : D + 1], 1.0)
        return qtr, ktr, vaug

    # ts goes through the Pool (SWDGE) ring, split into quarters, so the big
    # transfers run concurrently with the SP-ring q/k/v loads and the first
    # chunks arrive in time for the first DVE multiplies.
    TSPLIT = 4
    def _load_ts(g):
        tst = tst_pool.tile([128, NKV, SGRP], BF, tag="tst")
        step = NKV // TSPLIT
        for c in range(TSPLIT):
            nc.gpsimd.dma_start(
                tst[:, c * step : (c + 1) * step, :],
                ts[g, :, c * step : (c + 1) * step, :],
            )
        return tst

    tst0 = _load_ts(0)
    qtr0, ktr0, vaug0 = _load_head(0)
    qtr1, ktr1, vaug1 = _load_head(1)
    tst1 = _load_ts(1)
    qtrs, ktrs, vaugs = [qtr0, qtr1], [ktr0, ktr1], [vaug0, vaug1]
    tsts = [tst0, tst1]

    # ---------------- self-woven compute phases ------------------------------
    phases = [(h, g) for g in range(NGRP) for h in range(HPC)]

    class EvState:
        """EV accumulation for one phase, self-woven into that phase's own
        S-stream with a one-group lag (an et chunk's EV matmuls are emitted
        only after its exp+mul has been emitted).  J-outer order: the four
        s-block accumulators rotate over four half-bank PSUM tiles laid out
        so consecutive matmuls alternate banks (avoids the accumulate RMW
        stall); each block is normalized and stored when its J-loop closes."""

        def __init__(self, h, g, et):
            self.h, self.g, self.et = h, g, et
            self.pos = 0  # number of (J, b) steps emitted; J = pos//4, b = pos%4
            self.ops = [None] * NSBG

        def emit_upto(self, j_ready):
            """Emit EV matmuls for all chunks J < j_ready."""
            vaug = vaugs[self.h]
            while self.pos < 4 * j_ready:
                J, b = divmod(self.pos, 4)
                if J == 0:
                    self.ops[b] = ev_psum.tile(
                        [128, 256], FP, tag="evacc", name="evacc"
                    )
                nc.tensor.matmul(
                    self.ops[b][:, 0 : D + 1],
                    self.et[:, J, b * 128 : (b + 1) * 128],
                    vaug[:, J, :],
                    start=(J == 0), stop=(J == NKV - 1),
                )
                if J == NKV - 1:
                    sb = self.g * NSBG + b
                    recip = small_pool.tile([128, 1], FP, tag="recip")
                    nc.vector.reciprocal(recip[:], self.ops[b][:, D : D + 1])
                    ot = out_pool.tile([128, D], FP, tag="ot")
                    nc.vector.tensor_scalar_mul(ot[:], self.ops[b][:, 0:D], recip[:])
                    nc.sync.dma_start(
                        out[self.h, sb * 128 : (sb + 1) * 128, :], ot[:]
                    )
                self.pos += 1

        def finish(self):
            self.emit_upto(NKV)

    TRIP = 3
    groups = [TRIP] * (NKV // TRIP) + ([NKV % TRIP] if NKV % TRIP else [])

    prev = None  # EvState carrying the previous phase's small EV remainder
    for pi, (h, g) in enumerate(phases):
        qtr, ktr, tst = qtrs[h], ktrs[h], tsts[g]
        et = et_pool.tile([128, NKV, SGRP], BF, tag="et")
        sl = slice(g * SGRP, (g + 1) * SGRP)
        ev = EvState(h, g, et)
        J = 0
        for w in groups:
            sp = s_psum.tile([128, TRIP, SGRP], FP, tag="sps")
            for t in range(w):
                nc.tensor.matmul(
                    sp[:, t, :],
                    ktr[:, (J + t) * 128 : (J + t + 1) * 128],
                    qtr[:, sl],
                    start=True, stop=True,
                )
            # weave: drain the previous phase's EV remainder, then this
            # phase's own EV matmuls for every chunk already exp+mul'd
            if prev is not None:
                prev.finish()
                prev = None
            # lag two groups so the previous phase's accumulator norm reads
            # have cleared the PSUM slots this phase's J=0 matmuls reuse
            ev.emit_upto(max(0, J - TRIP))
            nc.scalar.activation(
                et[:, J : J + w, :], sp[:, 0:w, :],
                mybir.ActivationFunctionType.Exp,
            )
            nc.vector.tensor_mul(
                et[:, J : J + w, :], et[:, J : J + w, :], tst[:, J : J + w, :]
            )
            J += w
        prev = ev

    # tail: the last phase's final EV group runs unwoven (small)
    if prev is not None:
        prev.finish()


# ---------------------------------------------------------------------------
# Entry point: full unsharded inputs -> full output.
# Sharding: head-parallel, 2 heads per NeuronCore across 8 cores; the
# topk index/score tensors are shared by all cores.
# ---------------------------------------------------------------------------

_CACHE = {}


def make_in_maps(q, k, v, topk_indices, topk_scores, cfg):
    """Host-side prep: bf16 conversion, q pre-scaling + transpose, k
    transpose, dense TS table build.  Returns (in_maps, nmaxs)."""
    import ml_dtypes

    bf16 = ml_dtypes.bfloat16
    SQ, SKV, NKV, SGRP = cfg.SQ, cfg.SKV, cfg.NKV, cfg.SHALF
    NGRP = SQ // SGRP

    # dense TS[j, s] = sum of topk_scores over duplicate (s, j) selections
    idx = np.asarray(topk_indices)[0].astype(np.int64)          # [SQ, TOPK]
    sc = np.asarray(topk_scores, dtype=np.float32)[0]           # [SQ, TOPK]
    tsd = np.zeros((SKV, SQ), dtype=np.float32)                 # [j, s]
    s_arr = np.repeat(np.arange(SQ, dtype=np.int64), cfg.TOPK)
    np.add.at(tsd, (idx.reshape(-1), s_arr), sc.reshape(-1))
    # per group: [128, NKV, SGRP] with ts[p, J, s] = tsd[J*128 + p, g*SGRP + s]
    tsd = tsd.reshape(NKV, 128, NGRP, SGRP).transpose(2, 1, 0, 3)  # [g,p,J,s]
    ts_bf = np.ascontiguousarray(tsd.astype(bf16))

    qs = (np.asarray(q, dtype=np.float32) * (float(cfg.D) ** -0.5)).astype(bf16)
    kb = np.asarray(k, dtype=np.float32).astype(bf16)
    vb = np.asarray(v, dtype=np.float32).astype(bf16)
    qsT = np.ascontiguousarray(qs[0].transpose(0, 2, 1))  # [H, D, SQ]
    kbT = np.ascontiguousarray(kb[0].transpose(0, 2, 1))  # [H, D, SKV]

    in_maps = []
    for i in range(8):
        m = {
            "qT": np.ascontiguousarray(qsT[2 * i : 2 * i + 2]),
            "kT": np.ascontiguousarray(kbT[2 * i : 2 * i + 2]),
            "v": np.ascontiguousarray(vb[0, 2 * i : 2 * i + 2]),
            "ts": ts_bf,
        }
        in_maps.append(m)
    return in_maps, ()


def kernel(q, k, v, topk_indices, topk_scores):
    q = np.asarray(q, dtype=np.float32)
    B, H, SQ, D = q.shape
    SKV = np.asarray(k).shape[2]
    TOPK = np.asarray(topk_indices).shape[-1]
    assert B == 1 and H == 16 and SQ == 1024 and SKV == 4096 and D == 128

    cfg = Cfg(HPC=H // 8, SQ=SQ, SKV=SKV, D=D, TOPK=TOPK)
    in_maps, nmaxs = make_in_maps(q, k, v, topk_indices, topk_scores, cfg)

    nc = _CACHE.get("v3")
    if nc is None:
        nc = build_program(cfg, list(nmaxs), reps=1)
        _CACHE["v3"] = nc

    from concourse.bass_utils import run_bass_kernel_spmd

    res = run_bass_kernel_spmd(nc, in_maps, list(range(8)))
    out = np.stack([res.results[i]["out"] for i in range(8)])
    return out.reshape(1, H, SQ, D).astype(np.float32)


# revision 8
# speedup vs baseline: 1.1979x; 1.1979x over previous
"""DSA sparse attention (context-parallel variant) for Trainium2 via Bass/Tile.

Dense-rewrite algorithm (mathematically identical to the reference):
  w[s,t] = exp(sc[s,t])*ts[s,t] / sum_t' exp(sc)*ts   (softmax->*ts->renorm collapses)
  TS[s,j] = sum_t ts[s,t]*[idx[s,t]==j]  -> dense scatter of score values
  E[s,j]  = TS[s,j]*exp(scale*S[s,j]),  S = Q K^T (dense)
  O       = (E @ V) / rowsum(E)
Everything is computed in transposed layout (kv on partitions); O comes out
natural via E^T-stationary matmuls; rowsum(E) falls out of a ones-column
appended to V.

V3 layout/scheduling notes (over V2):
  - host pre-TRANSPOSES q (pre-scaled) and k to [D, S] layout: the on-chip
    PE transposes (80 matmuls) + DVE evacuation copies disappear and the
    q/k DMAs become perfectly contiguous per partition.
  - host pre-builds the DENSE TS table (bf16, [128, NKV, SGRP] per s-group)
    and the kernel DMAs it instead of running 64 gpsimd local_scatters:
    the Pool engine (47us scatters + 11us drains per rep) drops to zero.
  - S psum tiles are [128, 3, 512] (3 banks) so each ACT exp call covers
    1536 elements instead of 1024, amortizing the ~352-cycle ACT pipeline
    fill; EV accumulators shrink to 2 rotating full-bank tiles (the four
    s-blocks are processed in two half-phases of two blocks each), keeping
    total PSUM usage at exactly 8 banks.
  - phases run g-major: (h0,g0) (h1,g0) (h0,g1) (h1,g1); per phase the S^T
    matmuls are WOVEN with the EV matmuls of the previous phase so the PE
    alternates between ACT-gated S work and dependency-free EV work.
"""

import sys

sys.path.insert(0, "/opt/trn_rl_repo")

import numpy as np

import concourse.bass as bass
import concourse.bacc as bacc
import concourse.mybir as mybir
import concourse.tile as tile
from concourse.vector_clock import ScopedClock

# ---------------------------------------------------------------------------
# Patch: this walrus build encodes at most ONE sync-wait on a CTRL NO_STRUCT
# instruction; TileContext's tail drain carries one wait per live proc.  Split
# the waits across a chain of single-wait drains.
# ---------------------------------------------------------------------------


def _patched_drain_and_barrier(self, tick_clock, wait_clock):
    drain_inst = self.nc.sync.drain()
    wait_clock.add_sem_waits(
        drain_inst.ins, ScopedClock({None: tick_clock.global_clock})
    )
    si = drain_inst.ins.sync_info
    if si is not None and len(si.on_wait) > 1:
        waits = list(si.on_wait)
        drain_inst.ins.sync_info = mybir.SyncInfo(
            on_wait=waits[:1], on_update=list(si.on_update)
        )
        for i in range(1, len(waits)):
            extra = self.nc.sync.drain()
            extra.ins.sync_info = mybir.SyncInfo(on_wait=[waits[i]], on_update=[])
    self.nc.all_engine_barrier()
    assert self.sems is not None
    popped = self.nc._tile_sem_poison_stack.pop()
    assert popped is self._sem_poison
    self.nc.clear_and_free_semaphores(list(self.sems.allocated().values()))
    self.nc.all_engine_barrier()


tile.TileContext._drain_and_barrier = _patched_drain_and_barrier

FP = mybir.dt.float32
BF = mybir.dt.bfloat16


class Cfg:
    def __init__(self, HPC=2, SQ=1024, SKV=4096, D=128, TOPK=64):
        self.HPC = HPC  # heads per core
        self.SQ = SQ
        self.SKV = SKV
        self.D = D
        self.TOPK = TOPK
        self.NKV = SKV // 128  # kv chunks of 128
        self.NSB = SQ // 128  # query blocks of 128
        self.SHALF = 512  # s-group width (s-dim per group)
        self.scale = float(D) ** -0.5


# ---------------------------------------------------------------------------
# Program builder
# ---------------------------------------------------------------------------


def build_program(cfg, nmaxs=None, reps=1):
    nc = bacc.Bacc("TRN2", debug=False)
    HPC, SQ, SKV, D, NKV = cfg.HPC, cfg.SQ, cfg.SKV, cfg.D, cfg.NKV
    NGRP = SQ // cfg.SHALF

    qT = nc.dram_tensor("qT", [HPC, D, SQ], BF, kind="ExternalInput").ap()
    kT = nc.dram_tensor("kT", [HPC, D, SKV], BF, kind="ExternalInput").ap()
    v = nc.dram_tensor("v", [HPC, SKV, D], BF, kind="ExternalInput").ap()
    ts = nc.dram_tensor(
        "ts", [NGRP, 128, NKV, cfg.SHALF], BF, kind="ExternalInput"
    ).ap()
    out = nc.dram_tensor("out", [HPC, SQ, D], FP, kind="ExternalOutput").ap()

    with tile.TileContext(nc) as tc:
        import contextlib

        ctx = contextlib.ExitStack()
        with ctx:
            tst_pool = ctx.enter_context(tc.tile_pool(name="tst", bufs=2))
            ktr_pool = ctx.enter_context(tc.tile_pool(name="ktr", bufs=2))
            et_pool = ctx.enter_context(tc.tile_pool(name="et", bufs=2))
            small_pool = ctx.enter_context(tc.tile_pool(name="small", bufs=4))
            out_pool = ctx.enter_context(tc.tile_pool(name="outp", bufs=4))
            s_psum = ctx.enter_context(tc.tile_pool(name="sps", bufs=2, space="PSUM"))
            ev_psum = ctx.enter_context(tc.tile_pool(name="evp", bufs=2, space="PSUM"))

            def _body(_iv=None):
                _build_body(
                    nc, tc, cfg, qT, kT, v, ts, out,
                    tst_pool, ktr_pool, et_pool, small_pool, out_pool,
                    s_psum, ev_psum,
                )

            if reps == 1:
                _body()
            else:
                with tc.For_i(
                    0, reps, 1,
                    hint_engines=(
                        mybir.EngineType.PE,
                        mybir.EngineType.DVE,
                        mybir.EngineType.Activation,
                        mybir.EngineType.Pool,
                        mybir.EngineType.SP,
                    ),
                ):
                    _body()

    nc.compile()
    return nc


def _build_body(nc, tc, cfg, qT, kT, v, ts, out,
                tst_pool, ktr_pool, et_pool, small_pool, out_pool,
                s_psum, ev_psum):
    HPC, SQ, SKV, D, NKV = cfg.HPC, cfg.SQ, cfg.SKV, cfg.D, cfg.NKV
    SGRP = cfg.SHALF
    NGRP = SQ // SGRP
    NSBG = SGRP // 128  # s-blocks per group (4)
    STG = 16            # kv-chunks per v staging DMA

    # ---------------- input DMAs (contiguous, host-prepped layouts) ---------
    def _load_head(h):
        qtr = ktr_pool.tile([128, SQ], BF, tag="qtr")
        nc.sync.dma_start(qtr[:], qT[h])
        ktr = ktr_pool.tile([128, SKV], BF, tag="ktr")
        nc.sync.dma_start(ktr[:, 0 : SKV // 2], kT[h, :, 0 : SKV // 2])
        nc.sync.dma_start(ktr[:, SKV // 2 : SKV], kT[h, :, SKV // 2 : SKV])
        vaug = ktr_pool.tile([128, NKV, D + 1], BF, tag="vaug")
        vview = v[h].rearrange("(n p) d -> p n d", p=128)
        for gdma in range(NKV // STG):
            nc.sync.dma_start(
                vaug[:, gdma * STG : (gdma + 1) * STG, 0:D],
                vview[:, gdma * STG : (gdma + 1) * STG, :],
            )
        nc.vector.memset(vaug[:, :, D : D + 1], 1.0)
        return qtr, ktr, vaug

    # q/k/v issue first on the SP ring so the next rep's S matmuls never wait
    # behind the big ts transfers; the ts splits follow (their WAR stalls
    # resolve mid-rep, turning them into prefetches for the next rep).
    TSPLIT = 4
    def _load_ts(g):
        tst = tst_pool.tile([128, NKV, SGRP], BF, tag="tst")
        step = NKV // TSPLIT
        for c in range(TSPLIT):
            nc.sync.dma_start(
                tst[:, c * step : (c + 1) * step, :],
                ts[g, :, c * step : (c + 1) * step, :],
            )
        return tst

    qtr0, ktr0, vaug0 = _load_head(0)
    qtr1, ktr1, vaug1 = _load_head(1)
    tst0 = _load_ts(0)
    tst1 = _load_ts(1)
    qtrs, ktrs, vaugs = [qtr0, qtr1], [ktr0, ktr1], [vaug0, vaug1]
    tsts = [tst0, tst1]

    # ---------------- self-woven compute phases ------------------------------
    phases = [(h, g) for g in range(NGRP) for h in range(HPC)]

    class EvState:
        """EV accumulation for one phase, self-woven into that phase's own
        S-stream with a one-group lag (an et chunk's EV matmuls are emitted
        only after its exp+mul has been emitted).  J-outer order: the four
        s-block accumulators rotate over four half-bank PSUM tiles laid out
        so consecutive matmuls alternate banks (avoids the accumulate RMW
        stall); each block is normalized and stored when its J-loop closes."""

        def __init__(self, h, g, et):
            self.h, self.g, self.et = h, g, et
            self.pos = 0  # number of (J, b) steps emitted; J = pos//4, b = pos%4
            self.ops = [None] * NSBG

        def emit_upto(self, j_ready):
            """Emit EV matmuls for all chunks J < j_ready."""
            vaug = vaugs[self.h]
            while self.pos < 4 * j_ready:
                J, b = divmod(self.pos, 4)
                if J == 0:
                    self.ops[b] = ev_psum.tile(
                        [128, 256], FP, tag="evacc", name="evacc"
                    )
                nc.tensor.matmul(
                    self.ops[b][:, 0 : D + 1],
                    self.et[:, J, b * 128 : (b + 1) * 128],
                    vaug[:, J, :],
                    start=(J == 0), stop=(J == NKV - 1),
                )
                if J == NKV - 1:
                    sb = self.g * NSBG + b
                    recip = small_pool.tile([128, 1], FP, tag="recip")
                    nc.vector.reciprocal(recip[:], self.ops[b][:, D : D + 1])
                    ot = out_pool.tile([128, D], FP, tag="ot")
                    nc.vector.tensor_scalar_mul(ot[:], self.ops[b][:, 0:D], recip[:])
                    # outputs go through the Pool SWDGE ring so they never
                    # block input DMAs queued on the SP ring
                    nc.gpsimd.dma_start(
                        out[self.h, sb * 128 : (sb + 1) * 128, :], ot[:]
                    )
                self.pos += 1

        def finish(self):
            self.emit_upto(NKV)

    TRIP = 3
    groups = [TRIP] * (NKV // TRIP) + ([NKV % TRIP] if NKV % TRIP else [])

    prev = None  # EvState carrying the previous phase's small EV remainder
    for pi, (h, g) in enumerate(phases):
        qtr, ktr, tst = qtrs[h], ktrs[h], tsts[g]
        et = et_pool.tile([128, NKV, SGRP], BF, tag="et")
        sl = slice(g * SGRP, (g + 1) * SGRP)
        ev = EvState(h, g, et)
        J = 0
        for w in groups:
            sp = s_psum.tile([128, TRIP, SGRP], FP, tag="sps")
            for t in range(w):
                nc.tensor.matmul(
                    sp[:, t, :],
                    ktr[:, (J + t) * 128 : (J + t + 1) * 128],
                    qtr[:, sl],
                    start=True, stop=True,
                )
            # weave: drain the previous phase's EV remainder, then this
            # phase's own EV matmuls for every chunk already exp+mul'd
            if prev is not None:
                prev.finish()
                prev = None
            # lag two groups so the previous phase's accumulator norm reads
            # have cleared the PSUM slots this phase's J=0 matmuls reuse
            ev.emit_upto(max(0, J - TRIP))
            nc.scalar.activation(
                et[:, J : J + w, :], sp[:, 0:w, :],
                mybir.ActivationFunctionType.Exp,
            )
            nc.vector.tensor_mul(
                et[:, J : J + w, :], et[:, J : J + w, :], tst[:, J : J + w, :]
            )
            J += w
        prev = ev

    # tail: the last phase's final EV group runs unwoven (small)
    if prev is not None:
        prev.finish()


# ---------------------------------------------------------------------------
# Entry point: full unsharded inputs -> full output.
# Sharding: head-parallel, 2 heads per NeuronCore across 8 cores; the
# topk index/score tensors are shared by all cores.
# ---------------------------------------------------------------------------

_CACHE = {}


def make_in_maps(q, k, v, topk_indices, topk_scores, cfg):
    """Host-side prep: bf16 conversion, q pre-scaling + transpose, k
    transpose, dense TS table build.  Returns (in_maps, nmaxs)."""
    import ml_dtypes

    bf16 = ml_dtypes.bfloat16
    SQ, SKV, NKV, SGRP = cfg.SQ, cfg.SKV, cfg.NKV, cfg.SHALF
    NGRP = SQ // SGRP

    # dense TS[j, s] = sum of topk_scores over duplicate (s, j) selections
    idx = np.asarray(topk_indices)[0].astype(np.int64)          # [SQ, TOPK]
    sc = np.asarray(topk_scores, dtype=np.float32)[0]           # [SQ, TOPK]
    tsd = np.zeros((SKV, SQ), dtype=np.float32)                 # [j, s]
    s_arr = np.repeat(np.arange(SQ, dtype=np.int64), cfg.TOPK)
    np.add.at(tsd, (idx.reshape(-1), s_arr), sc.reshape(-1))
    # per group: [128, NKV, SGRP] with ts[p, J, s] = tsd[J*128 + p, g*SGRP + s]
    tsd = tsd.reshape(NKV, 128, NGRP, SGRP).transpose(2, 1, 0, 3)  # [g,p,J,s]
    ts_bf = np.ascontiguousarray(tsd.astype(bf16))

    qs = (np.asarray(q, dtype=np.float32) * (float(cfg.D) ** -0.5)).astype(bf16)
    kb = np.asarray(k, dtype=np.float32).astype(bf16)
    vb = np.asarray(v, dtype=np.float32).astype(bf16)
    qsT = np.ascontiguousarray(qs[0].transpose(0, 2, 1))  # [H, D, SQ]
    kbT = np.ascontiguousarray(kb[0].transpose(0, 2, 1))  # [H, D, SKV]

    in_maps = []
    for i in range(8):
        m = {
            "qT": np.ascontiguousarray(qsT[2 * i : 2 * i + 2]),
            "kT": np.ascontiguousarray(kbT[2 * i : 2 * i + 2]),
            "v": np.ascontiguousarray(vb[0, 2 * i : 2 * i + 2]),
            "ts": ts_bf,
        }
        in_maps.append(m)
    return in_maps, ()


def kernel(q, k, v, topk_indices, topk_scores):
    q = np.asarray(q, dtype=np.float32)
    B, H, SQ, D = q.shape
    SKV = np.asarray(k).shape[2]
    TOPK = np.asarray(topk_indices).shape[-1]
    assert B == 1 and H == 16 and SQ == 1024 and SKV == 4096 and D == 128

    cfg = Cfg(HPC=H // 8, SQ=SQ, SKV=SKV, D=D, TOPK=TOPK)
    in_maps, nmaxs = make_in_maps(q, k, v, topk_indices, topk_scores, cfg)

    nc = _CACHE.get("v3")
    if nc is None:
        nc = build_program(cfg, list(nmaxs), reps=1)
        _CACHE["v3"] = nc

    from concourse.bass_utils import run_bass_kernel_spmd

    res = run_bass_kernel_spmd(nc, in_maps, list(range(8)))
    out = np.stack([res.results[i]["out"] for i in range(8)])
    return out.reshape(1, H, SQ, D).astype(np.float32)


# revision 12
# speedup vs baseline: 1.4843x; 1.2391x over previous
"""DSA sparse attention (context-parallel variant) for Trainium2 via Bass/Tile.

Dense-rewrite algorithm (mathematically identical to the reference):
  w[s,t] = exp(sc[s,t])*ts[s,t] / sum_t' exp(sc)*ts   (softmax->*ts->renorm collapses)
  TS[s,j] = sum_t ts[s,t]*[idx[s,t]==j]  -> dense scatter of score values
  E[s,j]  = TS[s,j]*exp(scale*S[s,j]),  S = Q K^T (dense)
  O       = (E @ V) / rowsum(E)
Everything is computed in transposed layout (kv on partitions); O comes out
natural via E^T-stationary matmuls; rowsum(E) falls out of a ones-column
appended to V.

V3 layout/scheduling notes (over V2):
  - host pre-TRANSPOSES q (pre-scaled) and k to [D, S] layout: the on-chip
    PE transposes (80 matmuls) + DVE evacuation copies disappear and the
    q/k DMAs become perfectly contiguous per partition.
  - host pre-builds the DENSE TS table (bf16, [128, NKV, SGRP] per s-group)
    and the kernel DMAs it instead of running 64 gpsimd local_scatters:
    the Pool engine (47us scatters + 11us drains per rep) drops to zero.
  - S psum tiles are [128, 3, 512] (3 banks) so each ACT exp call covers
    1536 elements instead of 1024, amortizing the ~352-cycle ACT pipeline
    fill; EV accumulators shrink to 2 rotating full-bank tiles (the four
    s-blocks are processed in two half-phases of two blocks each), keeping
    total PSUM usage at exactly 8 banks.
  - phases run g-major: (h0,g0) (h1,g0) (h0,g1) (h1,g1); per phase the S^T
    matmuls are WOVEN with the EV matmuls of the previous phase so the PE
    alternates between ACT-gated S work and dependency-free EV work.
"""

import sys

sys.path.insert(0, "/opt/trn_rl_repo")

import numpy as np

import concourse.bass as bass
import concourse.bacc as bacc
import concourse.mybir as mybir
import concourse.tile as tile
from concourse.vector_clock import ScopedClock

# ---------------------------------------------------------------------------
# Patch: this walrus build encodes at most ONE sync-wait on a CTRL NO_STRUCT
# instruction; TileContext's tail drain carries one wait per live proc.  Split
# the waits across a chain of single-wait drains.
# ---------------------------------------------------------------------------


def _patched_drain_and_barrier(self, tick_clock, wait_clock):
    drain_inst = self.nc.sync.drain()
    wait_clock.add_sem_waits(
        drain_inst.ins, ScopedClock({None: tick_clock.global_clock})
    )
    si = drain_inst.ins.sync_info
    if si is not None and len(si.on_wait) > 1:
        waits = list(si.on_wait)
        drain_inst.ins.sync_info = mybir.SyncInfo(
            on_wait=waits[:1], on_update=list(si.on_update)
        )
        for i in range(1, len(waits)):
            extra = self.nc.sync.drain()
            extra.ins.sync_info = mybir.SyncInfo(on_wait=[waits[i]], on_update=[])
    self.nc.all_engine_barrier()
    assert self.sems is not None
    popped = self.nc._tile_sem_poison_stack.pop()
    assert popped is self._sem_poison
    self.nc.clear_and_free_semaphores(list(self.sems.allocated().values()))
    self.nc.all_engine_barrier()


tile.TileContext._drain_and_barrier = _patched_drain_and_barrier

FP = mybir.dt.float32
BF = mybir.dt.bfloat16


class Cfg:
    def __init__(self, HPC=2, SQ=1024, SKV=4096, D=128, TOPK=64):
        self.HPC = HPC  # heads per core
        self.SQ = SQ
        self.SKV = SKV
        self.D = D
        self.TOPK = TOPK
        self.NKV = SKV // 128  # kv chunks of 128
        self.NSB = SQ // 128  # query blocks of 128
        self.SHALF = 512  # s-group width (s-dim per group)
        self.scale = float(D) ** -0.5


# ---------------------------------------------------------------------------
# Program builder
# ---------------------------------------------------------------------------


def build_program(cfg, nmaxs=None, reps=1):
    nc = bacc.Bacc("TRN2", debug=False)
    HPC, SQ, SKV, D, NKV = cfg.HPC, cfg.SQ, cfg.SKV, cfg.D, cfg.NKV
    NGRP = SQ // cfg.SHALF

    qT = nc.dram_tensor("qT", [HPC, D, SQ], BF, kind="ExternalInput").ap()
    kT = nc.dram_tensor("kT", [HPC, D, SKV], BF, kind="ExternalInput").ap()
    # v arrives with the ones column pre-appended by the host: contiguous DMA
    va = nc.dram_tensor("va", [HPC, SKV, D + 1], BF, kind="ExternalInput").ap()
    ts = nc.dram_tensor(
        "ts", [NGRP, 128, NKV, cfg.SHALF], BF, kind="ExternalInput"
    ).ap()
    out = nc.dram_tensor("out", [HPC, SQ, D], FP, kind="ExternalOutput").ap()

    with tile.TileContext(nc) as tc:
        import contextlib

        ctx = contextlib.ExitStack()
        with ctx:
            tst_pool = ctx.enter_context(tc.tile_pool(name="tst", bufs=2))
            ktr_pool = ctx.enter_context(tc.tile_pool(name="ktr", bufs=2))
            et_pool = ctx.enter_context(tc.tile_pool(name="et", bufs=2))
            small_pool = ctx.enter_context(tc.tile_pool(name="small", bufs=4))
            out_pool = ctx.enter_context(tc.tile_pool(name="outp", bufs=4))
            s_psum = ctx.enter_context(tc.tile_pool(name="sps", bufs=2, space="PSUM"))
            ev_psum = ctx.enter_context(tc.tile_pool(name="evp", bufs=2, space="PSUM"))

            def _body(_iv=None):
                _build_body(
                    nc, tc, cfg, qT, kT, va, ts, out,
                    tst_pool, ktr_pool, et_pool, small_pool, out_pool,
                    s_psum, ev_psum,
                )

            # The For_i seam is an all-engine rendezvous (~7us) plus a cold
            # DMA restart; unrolling several reps per iteration amortizes it
            # and lets the WAR-ordered input DMAs prefetch across segments.
            UNROLL = 4
            if reps == 1:
                _body()
            else:
                assert reps % UNROLL == 0
                with tc.For_i(
                    0, reps // UNROLL, 1,
                    hint_engines=(
                        mybir.EngineType.PE,
                        mybir.EngineType.DVE,
                        mybir.EngineType.Activation,
                        mybir.EngineType.Pool,
                        mybir.EngineType.SP,
                    ),
                ):
                    for _u in range(UNROLL):
                        _body()

    nc.compile()
    return nc


def _build_body(nc, tc, cfg, qT, kT, va, ts, out,
                tst_pool, ktr_pool, et_pool, small_pool, out_pool,
                s_psum, ev_psum):
    HPC, SQ, SKV, D, NKV = cfg.HPC, cfg.SQ, cfg.SKV, cfg.D, cfg.NKV
    SGRP = cfg.SHALF
    NGRP = SQ // SGRP
    NSBG = SGRP // 128  # s-blocks per group (4)

    # ---------------- input DMAs (contiguous, host-prepped layouts) ---------
    # All inputs load through the in-order SP ring; the issue order below is
    # sorted by when each tile's previous-segment readers finish (WAR clear
    # time), so the ring head never blocks a transfer that could have run.
    def _load_qk(h):
        qtr = ktr_pool.tile([128, SQ], BF, tag="qtr")
        nc.sync.dma_start(qtr[:], qT[h])
        ktr = ktr_pool.tile([128, SKV], BF, tag="ktr")
        nc.sync.dma_start(ktr[:, 0 : SKV // 2], kT[h, :, 0 : SKV // 2])
        nc.sync.dma_start(ktr[:, SKV // 2 : SKV], kT[h, :, SKV // 2 : SKV])
        return qtr, ktr

    def _load_v(h):
        vaug = ktr_pool.tile([128, NKV, D + 1], BF, tag="vaug")
        vview = va[h].rearrange("(n p) d -> p n d", p=128)
        for c in range(2):
            nc.sync.dma_start(
                vaug[:, c * NKV // 2 : (c + 1) * NKV // 2, :],
                vview[:, c * NKV // 2 : (c + 1) * NKV // 2, :],
            )
        return vaug

    TSPLIT = 4
    def _load_ts(g):
        tst = tst_pool.tile([128, NKV, SGRP], BF, tag="tst")
        step = NKV // TSPLIT
        for c in range(TSPLIT):
            nc.sync.dma_start(
                tst[:, c * step : (c + 1) * step, :],
                ts[g, :, c * step : (c + 1) * step, :],
            )
        return tst

    qtr0, ktr0 = _load_qk(0)
    qtr1, ktr1 = _load_qk(1)
    tst0 = _load_ts(0)
    vaug0 = _load_v(0)
    tst1 = _load_ts(1)
    vaug1 = _load_v(1)
    qtrs, ktrs, vaugs = [qtr0, qtr1], [ktr0, ktr1], [vaug0, vaug1]
    tsts = [tst0, tst1]

    # ---------------- self-woven compute phases ------------------------------
    phases = [(h, g) for g in range(NGRP) for h in range(HPC)]

    class EvState:
        """EV accumulation for one phase, self-woven into that phase's own
        S-stream with a one-group lag (an et chunk's EV matmuls are emitted
        only after its exp+mul has been emitted).  J-outer order: the four
        s-block accumulators rotate over four half-bank PSUM tiles laid out
        so consecutive matmuls alternate banks (avoids the accumulate RMW
        stall); each block is normalized and stored when its J-loop closes."""

        def __init__(self, h, g, et):
            self.h, self.g, self.et = h, g, et
            self.pos = 0  # number of (J, b) steps emitted; J = pos//4, b = pos%4
            self.ops = [None] * NSBG

        def emit_upto(self, j_ready):
            """Emit EV matmuls for all chunks J < j_ready."""
            vaug = vaugs[self.h]
            while self.pos < 4 * j_ready:
                J, b = divmod(self.pos, 4)
                if J == 0:
                    self.ops[b] = ev_psum.tile(
                        [128, 256], FP, tag="evacc", name="evacc"
                    )
                nc.tensor.matmul(
                    self.ops[b][:, 0 : D + 1],
                    self.et[:, J, b * 128 : (b + 1) * 128],
                    vaug[:, J, :],
                    start=(J == 0), stop=(J == NKV - 1),
                )
                if J == NKV - 1:
                    sb = self.g * NSBG + b
                    recip = small_pool.tile([128, 1], FP, tag="recip")
                    nc.vector.reciprocal(recip[:], self.ops[b][:, D : D + 1])
                    ot = out_pool.tile([128, D], FP, tag="ot")
                    nc.vector.tensor_scalar_mul(ot[:], self.ops[b][:, 0:D], recip[:])
                    # outputs go through the Pool SWDGE ring so they never
                    # block input DMAs queued on the SP ring
                    nc.gpsimd.dma_start(
                        out[self.h, sb * 128 : (sb + 1) * 128, :], ot[:]
                    )
                self.pos += 1

        def finish(self):
            self.emit_upto(NKV)

    TRIP = 3
    groups = [TRIP] * (NKV // TRIP) + ([NKV % TRIP] if NKV % TRIP else [])

    prev = None  # EvState carrying the previous phase's small EV remainder
    for pi, (h, g) in enumerate(phases):
        qtr, ktr, tst = qtrs[h], ktrs[h], tsts[g]
        et = et_pool.tile([128, NKV, SGRP], BF, tag="et")
        sl = slice(g * SGRP, (g + 1) * SGRP)
        ev = EvState(h, g, et)
        J = 0
        for w in groups:
            sp = s_psum.tile([128, TRIP, SGRP], FP, tag="sps")
            for t in range(w):
                nc.tensor.matmul(
                    sp[:, t, :],
                    ktr[:, (J + t) * 128 : (J + t + 1) * 128],
                    qtr[:, sl],
                    start=True, stop=True,
                )
            # weave: drain the previous phase's EV remainder, then this
            # phase's own EV matmuls for every chunk already exp+mul'd
            if prev is not None:
                prev.finish()
                prev = None
            # lag two groups so the previous phase's accumulator norm reads
            # have cleared the PSUM slots this phase's J=0 matmuls reuse
            ev.emit_upto(max(0, J - TRIP))
            nc.scalar.activation(
                et[:, J : J + w, :], sp[:, 0:w, :],
                mybir.ActivationFunctionType.Exp,
            )
            nc.vector.tensor_mul(
                et[:, J : J + w, :], et[:, J : J + w, :], tst[:, J : J + w, :]
            )
            J += w
        prev = ev

    # tail: the last phase's final EV group runs unwoven (small)
    if prev is not None:
        prev.finish()


# ---------------------------------------------------------------------------
# Entry point: full unsharded inputs -> full output.
# Sharding: head-parallel, 2 heads per NeuronCore across 8 cores; the
# topk index/score tensors are shared by all cores.
# ---------------------------------------------------------------------------

_CACHE = {}


def make_in_maps(q, k, v, topk_indices, topk_scores, cfg):
    """Host-side prep: bf16 conversion, q pre-scaling + transpose, k
    transpose, dense TS table build.  Returns (in_maps, nmaxs)."""
    import ml_dtypes

    bf16 = ml_dtypes.bfloat16
    SQ, SKV, NKV, SGRP = cfg.SQ, cfg.SKV, cfg.NKV, cfg.SHALF
    NGRP = SQ // SGRP

    # dense TS[j, s] = sum of topk_scores over duplicate (s, j) selections
    idx = np.asarray(topk_indices)[0].astype(np.int64)          # [SQ, TOPK]
    sc = np.asarray(topk_scores, dtype=np.float32)[0]           # [SQ, TOPK]
    tsd = np.zeros((SKV, SQ), dtype=np.float32)                 # [j, s]
    s_arr = np.repeat(np.arange(SQ, dtype=np.int64), cfg.TOPK)
    np.add.at(tsd, (idx.reshape(-1), s_arr), sc.reshape(-1))
    # per group: [128, NKV, SGRP] with ts[p, J, s] = tsd[J*128 + p, g*SGRP + s]
    tsd = tsd.reshape(NKV, 128, NGRP, SGRP).transpose(2, 1, 0, 3)  # [g,p,J,s]
    ts_bf = np.ascontiguousarray(tsd.astype(bf16))

    qs = (np.asarray(q, dtype=np.float32) * (float(cfg.D) ** -0.5)).astype(bf16)
    kb = np.asarray(k, dtype=np.float32).astype(bf16)
    qsT = np.ascontiguousarray(qs[0].transpose(0, 2, 1))  # [H, D, SQ]
    kbT = np.ascontiguousarray(kb[0].transpose(0, 2, 1))  # [H, D, SKV]
    # v with a ones column appended (feeds the rowsum via the EV matmul)
    H = qs.shape[1]
    va = np.ones((H, SKV, cfg.D + 1), dtype=bf16)
    va[:, :, 0 : cfg.D] = np.asarray(v, dtype=np.float32)[0].astype(bf16)

    in_maps = []
    for i in range(8):
        m = {
            "qT": np.ascontiguousarray(qsT[2 * i : 2 * i + 2]),
            "kT": np.ascontiguousarray(kbT[2 * i : 2 * i + 2]),
            "va": np.ascontiguousarray(va[2 * i : 2 * i + 2]),
            "ts": ts_bf,
        }
        in_maps.append(m)
    return in_maps, ()


def kernel(q, k, v, topk_indices, topk_scores):
    q = np.asarray(q, dtype=np.float32)
    B, H, SQ, D = q.shape
    SKV = np.asarray(k).shape[2]
    TOPK = np.asarray(topk_indices).shape[-1]
    assert B == 1 and H == 16 and SQ == 1024 and SKV == 4096 and D == 128

    cfg = Cfg(HPC=H // 8, SQ=SQ, SKV=SKV, D=D, TOPK=TOPK)
    in_maps, nmaxs = make_in_maps(q, k, v, topk_indices, topk_scores, cfg)

    nc = _CACHE.get("v3")
    if nc is None:
        nc = build_program(cfg, list(nmaxs), reps=1)
        _CACHE["v3"] = nc

    from concourse.bass_utils import run_bass_kernel_spmd

    res = run_bass_kernel_spmd(nc, in_maps, list(range(8)))
    out = np.stack([res.results[i]["out"] for i in range(8)])
    return out.reshape(1, H, SQ, D).astype(np.float32)


# revision 15
# speedup vs baseline: 1.6745x; 1.1281x over previous
"""DSA sparse attention (context-parallel variant) for Trainium2 via Bass/Tile.

Dense-rewrite algorithm (mathematically identical to the reference):
  w[s,t] = exp(sc[s,t])*ts[s,t] / sum_t' exp(sc)*ts   (softmax->*ts->renorm collapses)
  TS[s,j] = sum_t ts[s,t]*[idx[s,t]==j]  -> dense scatter of score values
  E[s,j]  = TS[s,j]*exp(scale*S[s,j]),  S = Q K^T (dense)
  O       = (E @ V) / rowsum(E)
Everything is computed in transposed layout (kv on partitions); O comes out
natural via E^T-stationary matmuls; rowsum(E) falls out of a ones-column
appended to V.

V3 layout/scheduling notes (over V2):
  - host pre-TRANSPOSES q (pre-scaled) and k to [D, S] layout: the on-chip
    PE transposes (80 matmuls) + DVE evacuation copies disappear and the
    q/k DMAs become perfectly contiguous per partition.
  - host pre-builds the DENSE TS table (bf16, [128, NKV, SGRP] per s-group)
    and the kernel DMAs it instead of running 64 gpsimd local_scatters:
    the Pool engine (47us scatters + 11us drains per rep) drops to zero.
  - S psum tiles are [128, 3, 512] (3 banks) so each ACT exp call covers
    1536 elements instead of 1024, amortizing the ~352-cycle ACT pipeline
    fill; EV accumulators shrink to 2 rotating full-bank tiles (the four
    s-blocks are processed in two half-phases of two blocks each), keeping
    total PSUM usage at exactly 8 banks.
  - phases run g-major: (h0,g0) (h1,g0) (h0,g1) (h1,g1); per phase the S^T
    matmuls are WOVEN with the EV matmuls of the previous phase so the PE
    alternates between ACT-gated S work and dependency-free EV work.
"""

import sys

sys.path.insert(0, "/opt/trn_rl_repo")

import numpy as np

import concourse.bass as bass
import concourse.bacc as bacc
import concourse.mybir as mybir
import concourse.tile as tile
from concourse.vector_clock import ScopedClock

# ---------------------------------------------------------------------------
# Patch: this walrus build encodes at most ONE sync-wait on a CTRL NO_STRUCT
# instruction; TileContext's tail drain carries one wait per live proc.  Split
# the waits across a chain of single-wait drains.
# ---------------------------------------------------------------------------


def _patched_drain_and_barrier(self, tick_clock, wait_clock):
    drain_inst = self.nc.sync.drain()
    wait_clock.add_sem_waits(
        drain_inst.ins, ScopedClock({None: tick_clock.global_clock})
    )
    si = drain_inst.ins.sync_info
    if si is not None and len(si.on_wait) > 1:
        waits = list(si.on_wait)
        drain_inst.ins.sync_info = mybir.SyncInfo(
            on_wait=waits[:1], on_update=list(si.on_update)
        )
        for i in range(1, len(waits)):
            extra = self.nc.sync.drain()
            extra.ins.sync_info = mybir.SyncInfo(on_wait=[waits[i]], on_update=[])
    self.nc.all_engine_barrier()
    assert self.sems is not None
    popped = self.nc._tile_sem_poison_stack.pop()
    assert popped is self._sem_poison
    self.nc.clear_and_free_semaphores(list(self.sems.allocated().values()))
    self.nc.all_engine_barrier()


tile.TileContext._drain_and_barrier = _patched_drain_and_barrier

FP = mybir.dt.float32
BF = mybir.dt.bfloat16


class Cfg:
    def __init__(self, HPC=2, SQ=1024, SKV=4096, D=128, TOPK=64):
        self.HPC = HPC  # heads per core
        self.SQ = SQ
        self.SKV = SKV
        self.D = D
        self.TOPK = TOPK
        self.NKV = SKV // 128  # kv chunks of 128
        self.NSB = SQ // 128  # query blocks of 128
        self.SHALF = 512  # s-group width (s-dim per group)
        self.scale = float(D) ** -0.5


# ---------------------------------------------------------------------------
# Program builder
# ---------------------------------------------------------------------------


def build_program(cfg, nmaxs=None, reps=1):
    nc = bacc.Bacc("TRN2", debug=False)
    HPC, SQ, SKV, D, NKV = cfg.HPC, cfg.SQ, cfg.SKV, cfg.D, cfg.NKV
    NGRP = SQ // cfg.SHALF

    qT = nc.dram_tensor("qT", [HPC, D, SQ], BF, kind="ExternalInput").ap()
    kT = nc.dram_tensor("kT", [HPC, D, SKV], BF, kind="ExternalInput").ap()
    # v arrives with the ones column pre-appended by the host: contiguous DMA
    va = nc.dram_tensor("va", [HPC, SKV, D + 1], BF, kind="ExternalInput").ap()
    ts = nc.dram_tensor(
        "ts", [NGRP, 128, NKV, cfg.SHALF], BF, kind="ExternalInput"
    ).ap()
    out = nc.dram_tensor("out", [HPC, SQ, D], FP, kind="ExternalOutput").ap()

    with tile.TileContext(nc) as tc:
        import contextlib

        ctx = contextlib.ExitStack()
        with ctx:
            tst_pool = ctx.enter_context(tc.tile_pool(name="tst", bufs=2))
            ktr_pool = ctx.enter_context(tc.tile_pool(name="ktr", bufs=2))
            et_pool = ctx.enter_context(tc.tile_pool(name="et", bufs=2))
            small_pool = ctx.enter_context(tc.tile_pool(name="small", bufs=4))
            out_pool = ctx.enter_context(tc.tile_pool(name="outp", bufs=4))
            s_psum = ctx.enter_context(tc.tile_pool(name="sps", bufs=2, space="PSUM"))
            ev_psum = ctx.enter_context(tc.tile_pool(name="evp", bufs=2, space="PSUM"))

            def _body(nseg):
                _build_segments(
                    nc, tc, cfg, qT, kT, va, ts, out,
                    tst_pool, ktr_pool, et_pool, small_pool, out_pool,
                    s_psum, ev_psum, nseg,
                )

            # The For_i seam is an all-engine rendezvous (~7us) plus a cold
            # DMA restart; unrolling several reps per iteration amortizes it
            # and lets the WAR-ordered input DMAs prefetch across segments.
            UNROLL = 4
            if reps == 1:
                _body(1)
            else:
                assert reps % UNROLL == 0
                with tc.For_i(
                    0, reps // UNROLL, 1,
                    hint_engines=(
                        mybir.EngineType.PE,
                        mybir.EngineType.DVE,
                        mybir.EngineType.Activation,
                        mybir.EngineType.Pool,
                        mybir.EngineType.SP,
                    ),
                ):
                    _body(UNROLL)

    nc.compile()
    return nc


def _build_segments(nc, tc, cfg, qT, kT, va, ts, out,
                    tst_pool, ktr_pool, et_pool, small_pool, out_pool,
                    s_psum, ev_psum, nseg):
    HPC, SQ, SKV, D, NKV = cfg.HPC, cfg.SQ, cfg.SKV, cfg.D, cfg.NKV
    SGRP = cfg.SHALF
    NGRP = SQ // SGRP
    NSBG = SGRP // 128  # s-blocks per group (4)

    # ---------------- input DMAs (contiguous, host-prepped layouts) ---------
    # All inputs load through the in-order SP ring; the issue order below is
    # sorted by when each tile's previous-segment readers finish (WAR clear
    # time), so the ring head never blocks a transfer that could have run.
    def _load_qk(h):
        qtr = ktr_pool.tile([128, SQ], BF, tag="qtr")
        nc.sync.dma_start(qtr[:], qT[h])
        ktr = ktr_pool.tile([128, SKV], BF, tag="ktr")
        nc.sync.dma_start(ktr[:, 0 : SKV // 2], kT[h, :, 0 : SKV // 2])
        nc.sync.dma_start(ktr[:, SKV // 2 : SKV], kT[h, :, SKV // 2 : SKV])
        return qtr, ktr

    def _load_v(h):
        vaug = ktr_pool.tile([128, NKV, D + 1], BF, tag="vaug")
        vview = va[h].rearrange("(n p) d -> p n d", p=128)
        for c in range(2):
            nc.sync.dma_start(
                vaug[:, c * NKV // 2 : (c + 1) * NKV // 2, :],
                vview[:, c * NKV // 2 : (c + 1) * NKV // 2, :],
            )
        return vaug

    TSPLIT = 4
    def _load_ts(g):
        tst = tst_pool.tile([128, NKV, SGRP], BF, tag="tst")
        step = NKV // TSPLIT
        for c in range(TSPLIT):
            nc.sync.dma_start(
                tst[:, c * step : (c + 1) * step, :],
                ts[g, :, c * step : (c + 1) * step, :],
            )
        return tst

    class EvState:
        """Pending EV accumulation for one completed phase, woven into the
        NEXT phase's S-stream (every dependency — exp+mul of its et — is
        long met by then, so the in-order PE queue never stalls on it).
        J-outer order: the four s-block accumulators rotate over four
        half-bank PSUM tiles laid out so consecutive matmuls alternate
        banks (avoids the accumulate RMW stall); each block is normalized
        and stored when its J-loop closes."""

        def __init__(self, h, g, et, vaug):
            self.h, self.g, self.et, self.vaug = h, g, et, vaug
            self.pos = 0  # number of (J, b) steps emitted; J = pos//4, b = pos%4
            self.ops = [None] * NSBG

        def emit(self, n):
            for _ in range(n):
                if self.pos >= 4 * NKV:
                    return
                J, b = divmod(self.pos, 4)
                if J == 0:
                    self.ops[b] = ev_psum.tile(
                        [128, 256], FP, tag="evacc", name="evacc"
                    )
                nc.tensor.matmul(
                    self.ops[b][:, 0 : D + 1],
                    self.et[:, J, b * 128 : (b + 1) * 128],
                    self.vaug[:, J, :],
                    start=(J == 0), stop=(J == NKV - 1),
                )
                if J == NKV - 1:
                    sb = self.g * NSBG + b
                    recip = small_pool.tile([128, 1], FP, tag="recip")
                    nc.vector.reciprocal(recip[:], self.ops[b][:, D : D + 1])
                    ot = out_pool.tile([128, D], FP, tag="ot")
                    nc.vector.tensor_scalar_mul(ot[:], self.ops[b][:, 0:D], recip[:])
                    # outputs go through the Pool SWDGE ring so they never
                    # block input DMAs queued on the SP ring
                    nc.gpsimd.dma_start(
                        out[self.h, sb * 128 : (sb + 1) * 128, :], ot[:]
                    )
                self.pos += 1

        def finish(self):
            self.emit(4 * NKV - self.pos)

    TRIP = 3
    groups = [TRIP] * (NKV // TRIP) + ([NKV % TRIP] if NKV % TRIP else [])
    nweave = -(-4 * NKV // len(groups))  # prev-EV matmuls per S-group

    phases = [(h, g) for g in range(NGRP) for h in range(HPC)]
    prev = None  # EvState of the phase whose EV is pending

    for seg in range(nseg):
        qtr0, ktr0 = _load_qk(0)
        qtr1, ktr1 = _load_qk(1)
        tst0 = _load_ts(0)
        vaug0 = _load_v(0)
        tst1 = _load_ts(1)
        vaug1 = _load_v(1)
        qtrs, ktrs, vaugs = [qtr0, qtr1], [ktr0, ktr1], [vaug0, vaug1]
        tsts = [tst0, tst1]

        for pi, (h, g) in enumerate(phases):
            qtr, ktr, tst = qtrs[h], ktrs[h], tsts[g]
            et = et_pool.tile([128, NKV, SGRP], BF, tag="et")
            sl = slice(g * SGRP, (g + 1) * SGRP)
            J = 0
            for w in groups:
                sp = s_psum.tile([128, TRIP, SGRP], FP, tag="sps")
                for t in range(w):
                    nc.tensor.matmul(
                        sp[:, t, :],
                        ktr[:, (J + t) * 128 : (J + t + 1) * 128],
                        qtr[:, sl],
                        start=True, stop=True,
                    )
                if prev is not None:
                    prev.emit(nweave)
                nc.scalar.activation(
                    et[:, J : J + w, :], sp[:, 0:w, :],
                    mybir.ActivationFunctionType.Exp,
                )
                nc.vector.tensor_mul(
                    et[:, J : J + w, :], et[:, J : J + w, :], tst[:, J : J + w, :]
                )
                J += w
            if prev is not None:
                prev.finish()
            prev = EvState(h, g, et, vaugs[h])

    # tail: the final phase's EV runs unwoven at the segment-block end
    if prev is not None:
        prev.finish()


# ---------------------------------------------------------------------------
# Entry point: full unsharded inputs -> full output.
# Sharding: head-parallel, 2 heads per NeuronCore across 8 cores; the
# topk index/score tensors are shared by all cores.
# ---------------------------------------------------------------------------

_CACHE = {}


def make_in_maps(q, k, v, topk_indices, topk_scores, cfg):
    """Host-side prep: bf16 conversion, q pre-scaling + transpose, k
    transpose, dense TS table build.  Returns (in_maps, nmaxs)."""
    import ml_dtypes

    bf16 = ml_dtypes.bfloat16
    SQ, SKV, NKV, SGRP = cfg.SQ, cfg.SKV, cfg.NKV, cfg.SHALF
    NGRP = SQ // SGRP

    # dense TS[j, s] = sum of topk_scores over duplicate (s, j) selections
    idx = np.asarray(topk_indices)[0].astype(np.int64)          # [SQ, TOPK]
    sc = np.asarray(topk_scores, dtype=np.float32)[0]           # [SQ, TOPK]
    tsd = np.zeros((SKV, SQ), dtype=np.float32)                 # [j, s]
    s_arr = np.repeat(np.arange(SQ, dtype=np.int64), cfg.TOPK)
    np.add.at(tsd, (idx.reshape(-1), s_arr), sc.reshape(-1))
    # per group: [128, NKV, SGRP] with ts[p, J, s] = tsd[J*128 + p, g*SGRP + s]
    tsd = tsd.reshape(NKV, 128, NGRP, SGRP).transpose(2, 1, 0, 3)  # [g,p,J,s]
    ts_bf = np.ascontiguousarray(tsd.astype(bf16))

    qs = (np.asarray(q, dtype=np.float32) * (float(cfg.D) ** -0.5)).astype(bf16)
    kb = np.asarray(k, dtype=np.float32).astype(bf16)
    qsT = np.ascontiguousarray(qs[0].transpose(0, 2, 1))  # [H, D, SQ]
    kbT = np.ascontiguousarray(kb[0].transpose(0, 2, 1))  # [H, D, SKV]
    # v with a ones column appended (feeds the rowsum via the EV matmul)
    H = qs.shape[1]
    va = np.ones((H, SKV, cfg.D + 1), dtype=bf16)
    va[:, :, 0 : cfg.D] = np.asarray(v, dtype=np.float32)[0].astype(bf16)

    in_maps = []
    for i in range(8):
        m = {
            "qT": np.ascontiguousarray(qsT[2 * i : 2 * i + 2]),
            "kT": np.ascontiguousarray(kbT[2 * i : 2 * i + 2]),
            "va": np.ascontiguousarray(va[2 * i : 2 * i + 2]),
            "ts": ts_bf,
        }
        in_maps.append(m)
    return in_maps, ()


def kernel(q, k, v, topk_indices, topk_scores):
    q = np.asarray(q, dtype=np.float32)
    B, H, SQ, D = q.shape
    SKV = np.asarray(k).shape[2]
    TOPK = np.asarray(topk_indices).shape[-1]
    assert B == 1 and H == 16 and SQ == 1024 and SKV == 4096 and D == 128

    cfg = Cfg(HPC=H // 8, SQ=SQ, SKV=SKV, D=D, TOPK=TOPK)
    in_maps, nmaxs = make_in_maps(q, k, v, topk_indices, topk_scores, cfg)

    nc = _CACHE.get("v3")
    if nc is None:
        nc = build_program(cfg, list(nmaxs), reps=1)
        _CACHE["v3"] = nc

    from concourse.bass_utils import run_bass_kernel_spmd

    res = run_bass_kernel_spmd(nc, in_maps, list(range(8)))
    out = np.stack([res.results[i]["out"] for i in range(8)])
    return out.reshape(1, H, SQ, D).astype(np.float32)
